# revision 3
# baseline (speedup 1.0000x reference)
"""DIEN GRU-with-attention kernel for Trainium2 (8 NeuronCores, Bass/Tile).

Math note: the reference computes softmax over a singleton axis, which is
exactly 1.0, so the attention branch (Wa, item) never affects the output.
The computation reduces to a plain GRU:
    u  = sigmoid(x_t @ Wu + h @ Uu + bu)
    r  = sigmoid(x_t @ Wr + h @ Ur + br)
    hh = tanh(x_t @ Wh + r * (h @ Uh) + bh)
    h' = (1 - u) * h + u * hh

Device layout is feature-major: tiles are [D=128 partitions, batch free].
The batch dim (2048) is sharded 8 ways (256 per core); the host does the
[batch, feat] <-> [feat, batch] layout transposes during shard/unshard.
"""

import sys

if "/opt/trn_rl_repo" not in sys.path:
    sys.path.insert(0, "/opt/trn_rl_repo")

from concurrent.futures import ThreadPoolExecutor
from contextlib import ExitStack

import numpy as np

B, S, D = 2048, 200, 128
NCORES = 8
BS = B // NCORES  # batch per core

_BUILT = None  # cached compiled module


def _body(ctx, tc, aps, n_steps):
    import concourse.bass as bass  # noqa: F401
    from concourse import mybir

    nc = tc.nc
    f32 = mybir.dt.float32
    Sigmoid = mybir.ActivationFunctionType.Sigmoid
    Tanh = mybir.ActivationFunctionType.Tanh

    singles = ctx.enter_context(tc.tile_pool(name="singles", bufs=1))
    xpool = ctx.enter_context(tc.tile_pool(name="xp", bufs=4))
    hpool = ctx.enter_context(tc.tile_pool(name="hp", bufs=3))
    tmp = ctx.enter_context(tc.tile_pool(name="tmp", bufs=3))
    p_ur_pool = ctx.enter_context(tc.tile_pool(name="p_ur", bufs=2, space="PSUM"))
    p_g_pool = ctx.enter_context(tc.tile_pool(name="p_g", bufs=2, space="PSUM"))

    W = {}
    for name in ("Wu", "Wr", "Wh", "Uu", "Ur", "Uh", "ident"):
        t = singles.tile([D, D], f32, tag=name)
        nc.sync.dma_start(t[:], aps[name])
        W[name] = t
    Bv = {}
    for name in ("bu", "br", "bh"):
        t = singles.tile([D, 1], f32, tag=name)
        nc.sync.dma_start(t[:], aps[name])
        Bv[name] = t

    h = hpool.tile([D, BS], f32, tag="h")
    nc.sync.dma_start(h[:], aps["h0T"])

    xT = aps["xT"]
    outT = aps["outT"]

    for t_step in range(n_steps):
        xt = xpool.tile([D, BS], f32, tag="x")
        nc.sync.dma_start(xt[:], xT[t_step])

        p_u = p_ur_pool.tile([D, BS], f32, tag="pu")
        p_r = p_ur_pool.tile([D, BS], f32, tag="pr")
        p_zh = p_g_pool.tile([D, BS], f32, tag="zh")
        p_hh = p_g_pool.tile([D, BS], f32, tag="hh")

        # x projections (independent of h)
        nc.tensor.matmul(p_u[:], W["Wu"][:], xt[:], start=True, stop=False)
        nc.tensor.matmul(p_r[:], W["Wr"][:], xt[:], start=True, stop=False)
        nc.tensor.matmul(p_zh[:], W["Wh"][:], xt[:], start=True, stop=False)
        # h projections
        nc.tensor.matmul(p_u[:], W["Uu"][:], h[:], start=False, stop=True)
        nc.tensor.matmul(p_r[:], W["Ur"][:], h[:], start=False, stop=True)
        nc.tensor.matmul(p_hh[:], W["Uh"][:], h[:], start=True, stop=True)

        u = tmp.tile([D, BS], f32, tag="u")
        r = tmp.tile([D, BS], f32, tag="r")
        nc.scalar.activation(u[:], p_u[:], Sigmoid, bias=Bv["bu"][:])
        nc.scalar.activation(r[:], p_r[:], Sigmoid, bias=Bv["br"][:])

        m = tmp.tile([D, BS], f32, tag="m")
        nc.vector.tensor_mul(m[:], r[:], p_hh[:])
        # accumulate m into p_zh on the PE (identity matmul) instead of a DVE add
        nc.tensor.matmul(p_zh[:], W["ident"][:], m[:], start=False, stop=True)

        hh = tmp.tile([D, BS], f32, tag="hh")
        nc.scalar.activation(hh[:], p_zh[:], Tanh, bias=Bv["bh"][:])

        d = tmp.tile([D, BS], f32, tag="d")
        nc.vector.tensor_sub(d[:], hh[:], h[:])
        e = tmp.tile([D, BS], f32, tag="e")
        nc.vector.tensor_mul(e[:], u[:], d[:])
        h_new = hpool.tile([D, BS], f32, tag="h")
        nc.vector.tensor_add(h_new[:], h[:], e[:])

        nc.sync.dma_start(outT[t_step], h_new[:])
        h = h_new


def build_module(n_steps=S):
    import concourse.bacc as bacc
    import concourse.tile as tile
    from concourse import mybir

    f32 = mybir.dt.float32
    nc = bacc.Bacc(
        "TRN2",
        target_bir_lowering=False,
        debug=False,
        enable_asserts=False,
        num_devices=NCORES,
    )

    aps = {}
    aps["xT"] = nc.dram_tensor("xT", [n_steps, D, BS], f32, kind="ExternalInput").ap()
    aps["h0T"] = nc.dram_tensor("h0T", [D, BS], f32, kind="ExternalInput").ap()
    for name in ("Wu", "Wr", "Wh", "Uu", "Ur", "Uh", "ident"):
        aps[name] = nc.dram_tensor(name, [D, D], f32, kind="ExternalInput").ap()
    for name in ("bu", "br", "bh"):
        aps[name] = nc.dram_tensor(name, [D, 1], f32, kind="ExternalInput").ap()
    aps["outT"] = nc.dram_tensor(
        "outT", [n_steps, D, BS], f32, kind="ExternalOutput"
    ).ap()

    with tile.TileContext(nc) as tc, ExitStack() as ctx:
        _body(ctx, tc, aps, n_steps)
    nc.compile()
    return nc


def _get_built():
    global _BUILT
    if _BUILT is None:
        _BUILT = build_module(S)
    return _BUILT


def _shard_core(c, x, h0):
    xc = x[c * BS : (c + 1) * BS]  # [BS, S, D]
    xT = np.empty((S, D, BS), dtype=np.float32)
    for t in range(S):
        xT[t] = xc[:, t, :].T
    h0T = np.ascontiguousarray(h0[c * BS : (c + 1) * BS].T)
    return xT, h0T


def _prep_in_maps(inputs):
    x = np.ascontiguousarray(np.asarray(inputs["x"], dtype=np.float32))
    h0 = np.ascontiguousarray(np.asarray(inputs["h0"], dtype=np.float32))
    shared = {}
    for name in ("Wu", "Wr", "Wh", "Uu", "Ur", "Uh"):
        shared[name] = np.ascontiguousarray(np.asarray(inputs[name], dtype=np.float32))
    for name in ("bu", "br", "bh"):
        shared[name] = np.ascontiguousarray(
            np.asarray(inputs[name], dtype=np.float32).reshape(1, D).T
        )
    shared["ident"] = np.eye(D, dtype=np.float32)

    with ThreadPoolExecutor(NCORES) as ex:
        parts = list(ex.map(lambda c: _shard_core(c, x, h0), range(NCORES)))

    in_maps = []
    for c in range(NCORES):
        xT, h0T = parts[c]
        m = {"xT": xT, "h0T": h0T}
        m.update(shared)
        in_maps.append(m)
    return in_maps


def _unshard_core(c, outT, outs):
    # outT: [S, D, BS] -> outs[c*BS:(c+1)*BS] = [BS, S, D]
    dst = outs[c * BS : (c + 1) * BS]
    for t in range(S):
        dst[:, t, :] = outT[t].T


def _assemble(results):
    outs = np.empty((B, S, D), dtype=np.float32)
    with ThreadPoolExecutor(NCORES) as ex:
        list(
            ex.map(
                lambda c: _unshard_core(c, results[c]["outT"], outs), range(NCORES)
            )
        )
    h_last = np.ascontiguousarray(outs[:, -1, :])
    return outs, h_last


def _ensure_ntff_hook():
    """Install the axon NTFF profile hook if the image's antenv lacks it."""
    try:
        from antenv.axon_hooks import get_axon_ntff_profile_hook  # noqa: F401

        return True
    except ImportError:
        pass
    try:
        import types

        import antenv
        from trn_agent_boot.trn_boot import _ntff_profile_via_ctypes

        hook = _ntff_profile_via_ctypes("/opt/axon/libaxon_pjrt.so")
        mod = types.ModuleType("antenv.axon_hooks")
        mod.get_axon_ntff_profile_hook = lambda: hook
        mod.set_axon_ntff_profile_hook = lambda h: None
        sys.modules["antenv.axon_hooks"] = mod
        antenv.axon_hooks = mod
        return hook is not None
    except Exception as e:  # pragma: no cover
        print(f"NTFF hook install failed: {e}", file=sys.stderr)
        return False


def run(inputs, trace=False):
    """Run on hardware; returns ((outs, h_last), exec_time_ns_or_None)."""
    import concourse.bass_utils as bass_utils

    if trace:
        _ensure_ntff_hook()
        bass_utils.upload_artifacts = lambda tmpdir: f"local:{tmpdir}"

    nc = _get_built()
    in_maps = _prep_in_maps(inputs)
    res = bass_utils.run_bass_kernel_spmd(
        nc, in_maps, core_ids=list(range(NCORES)), trace=trace
    )
    return _assemble(res.results), res.exec_time_ns


def kernel(**inputs):
    (outs, h_last), _ = run(inputs, trace=False)
    return outs, h_last


# revision 17
# speedup vs baseline: 1.2364x; 1.2364x over previous
"""DIEN GRU-with-attention kernel for Trainium2 (8 NeuronCores, Bass/Tile).

Math note: the reference computes softmax over a singleton axis, which is
exactly 1.0, so the attention branch (Wa, item) never affects the output.
The computation reduces to a plain GRU:
    u  = sigmoid(x_t @ Wu + h @ Uu + bu)
    r  = sigmoid(x_t @ Wr + h @ Ur + br)
    hh = tanh(x_t @ Wh + r * (h @ Uh) + bh)
    h' = (1 - u) * h + u * hh

Device layout is feature-major: tiles are [D=128 partitions, batch free].
The batch dim (2048) is sharded 8 ways (256 per core); the host does the
[batch, feat] <-> [feat, batch] layout transposes during shard/unshard.
"""

import sys

if "/opt/trn_rl_repo" not in sys.path:
    sys.path.insert(0, "/opt/trn_rl_repo")

from concurrent.futures import ThreadPoolExecutor
from contextlib import ExitStack

import os

import numpy as np

B, S, D = 2048, 200, 128
NCORES = 8
BS = B // NCORES  # batch per core

# float32r streams the PE at 1 cycle/row (vs 4 for fp32's two-pass lowering)
# but measured 5.6e-2 rel err end-to-end — too lossy. Default off.
USE_F32R = os.environ.get("DIEN_F32R", "0") == "1"

_BUILT = None  # cached compiled module


def _body(ctx, tc, aps, n_steps):
    import concourse.bass as bass  # noqa: F401
    from concourse import mybir

    nc = tc.nc
    f32 = mybir.dt.float32
    Sigmoid = mybir.ActivationFunctionType.Sigmoid
    Tanh = mybir.ActivationFunctionType.Tanh

    fmm = mybir.dt.float32r if USE_F32R else f32

    singles = ctx.enter_context(tc.tile_pool(name="singles", bufs=1))
    xpool = ctx.enter_context(tc.tile_pool(name="xp", bufs=4))
    hpool = ctx.enter_context(tc.tile_pool(name="hp", bufs=3))
    tmp = ctx.enter_context(tc.tile_pool(name="tmp", bufs=3))
    p_ur_pool = ctx.enter_context(tc.tile_pool(name="p_ur", bufs=2, space="PSUM"))
    p_g_pool = ctx.enter_context(tc.tile_pool(name="p_g", bufs=2, space="PSUM"))

    W = {}
    for name in ("Wu", "Wr", "Wh", "Uu", "Ur", "Uh"):
        t = singles.tile([D, D], fmm, tag=name)
        nc.sync.dma_start(t[:], aps[name])
        W[name] = t
    Bv = {}
    for name in ("bu", "br", "bh", "nbu"):
        t = singles.tile([D, 1], f32, tag=name)
        nc.sync.dma_start(t[:], aps[name])
        Bv[name] = t

    h = hpool.tile([D, BS], fmm, tag="h")
    nc.sync.dma_start(h[:], aps["h0T"])

    xT = aps["xT"]
    outT = aps["outT"]

    for t_step in range(n_steps):
        xt = xpool.tile([D, BS], fmm, tag="x")
        nc.sync.dma_start(xt[:], xT[t_step])

        p_u = p_ur_pool.tile([D, BS], f32, tag="pu")
        p_r = p_ur_pool.tile([D, BS], f32, tag="pr")
        p_zh = p_g_pool.tile([D, BS], f32, tag="zh")
        p_hh = p_g_pool.tile([D, BS], f32, tag="hh")

        # x projections (independent of h; start each PSUM group, so they
        # must execute before the matching h-matmul accumulate)
        nc.tensor.matmul(p_r[:], W["Wr"][:], xt[:], start=True, stop=False)
        nc.tensor.matmul(p_zh[:], W["Wh"][:], xt[:], start=True, stop=True)
        nc.tensor.matmul(p_u[:], W["Wu"][:], xt[:], start=True, stop=False)
        # h projections: mm_hr heads the serial chain (sigma_r -> m -> z ->
        # tanh), so issue it first once h is ready; mm_hh feeds m next.
        nc.tensor.matmul(p_r[:], W["Ur"][:], h[:], start=False, stop=True)
        nc.tensor.matmul(p_hh[:], W["Uh"][:], h[:], start=True, stop=True)
        nc.tensor.matmul(p_u[:], W["Uu"][:], h[:], start=False, stop=True)

        r = tmp.tile([D, BS], f32, tag="r")
        nc.scalar.activation(r[:], p_r[:], Sigmoid, bias=Bv["br"][:])
        # um = 1 - u = sigmoid(-(zu + bu)); same PSUM bank, negated scale/bias
        um = tmp.tile([D, BS], f32, tag="um")
        nc.scalar.activation(um[:], p_u[:], Sigmoid, bias=Bv["nbu"][:], scale=-1.0)
        u = tmp.tile([D, BS], f32, tag="u")
        nc.scalar.activation(u[:], p_u[:], Sigmoid, bias=Bv["bu"][:])

        m = tmp.tile([D, BS], f32, tag="m")
        nc.vector.tensor_mul(m[:], r[:], p_hh[:])
        z = tmp.tile([D, BS], f32, tag="z")
        nc.vector.tensor_add(z[:], m[:], p_zh[:])

        hh = tmp.tile([D, BS], f32, tag="hh")
        nc.scalar.activation(hh[:], z[:], Tanh, bias=Bv["bh"][:])

        # blend: h' = (1-u)*h + u*hh; q1 runs off the tanh chain
        q1 = tmp.tile([D, BS], f32, tag="q1")
        nc.vector.tensor_mul(q1[:], um[:], h[:])
        q2 = tmp.tile([D, BS], f32, tag="q2")
        nc.vector.tensor_mul(q2[:], u[:], hh[:])
        h_new = hpool.tile([D, BS], fmm, tag="h")
        nc.vector.tensor_add(h_new[:], q1[:], q2[:])

        nc.sync.dma_start(outT[t_step], h_new[:])
        h = h_new


def build_module(n_steps=S):
    import concourse.bacc as bacc
    import concourse.tile as tile
    from concourse import mybir

    f32 = mybir.dt.float32
    fmm = mybir.dt.float32r if USE_F32R else f32
    nc = bacc.Bacc(
        "TRN2",
        target_bir_lowering=False,
        debug=False,
        enable_asserts=False,
        num_devices=NCORES,
    )

    aps = {}
    aps["xT"] = nc.dram_tensor("xT", [n_steps, D, BS], fmm, kind="ExternalInput").ap()
    aps["h0T"] = nc.dram_tensor("h0T", [D, BS], fmm, kind="ExternalInput").ap()
    for name in ("Wu", "Wr", "Wh", "Uu", "Ur", "Uh"):
        aps[name] = nc.dram_tensor(name, [D, D], fmm, kind="ExternalInput").ap()
    for name in ("bu", "br", "bh", "nbu"):
        aps[name] = nc.dram_tensor(name, [D, 1], f32, kind="ExternalInput").ap()
    aps["outT"] = nc.dram_tensor(
        "outT", [n_steps, D, BS], fmm, kind="ExternalOutput"
    ).ap()

    with tile.TileContext(nc) as tc, ExitStack() as ctx:
        _body(ctx, tc, aps, n_steps)
    nc.compile()
    return nc


def _get_built():
    global _BUILT
    if _BUILT is None:
        _BUILT = build_module(S)
    return _BUILT


def _shard_core(c, x, h0):
    xc = x[c * BS : (c + 1) * BS]  # [BS, S, D]
    xT = np.empty((S, D, BS), dtype=np.float32)
    for t in range(S):
        xT[t] = xc[:, t, :].T
    h0T = np.ascontiguousarray(h0[c * BS : (c + 1) * BS].T)
    return xT, h0T


def _prep_in_maps(inputs):
    x = np.ascontiguousarray(np.asarray(inputs["x"], dtype=np.float32))
    h0 = np.ascontiguousarray(np.asarray(inputs["h0"], dtype=np.float32))
    shared = {}
    for name in ("Wu", "Wr", "Wh", "Uu", "Ur", "Uh"):
        shared[name] = np.ascontiguousarray(np.asarray(inputs[name], dtype=np.float32))
    for name in ("bu", "br", "bh"):
        shared[name] = np.ascontiguousarray(
            np.asarray(inputs[name], dtype=np.float32).reshape(1, D).T
        )
    shared["nbu"] = np.ascontiguousarray(-shared["bu"])
    with ThreadPoolExecutor(NCORES) as ex:
        parts = list(ex.map(lambda c: _shard_core(c, x, h0), range(NCORES)))

    in_maps = []
    for c in range(NCORES):
        xT, h0T = parts[c]
        m = {"xT": xT, "h0T": h0T}
        m.update(shared)
        in_maps.append(m)
    return in_maps


def _unshard_core(c, outT, outs):
    # outT: [S, D, BS] -> outs[c*BS:(c+1)*BS] = [BS, S, D]
    dst = outs[c * BS : (c + 1) * BS]
    for t in range(S):
        dst[:, t, :] = outT[t].T


def _assemble(results):
    outs = np.empty((B, S, D), dtype=np.float32)
    with ThreadPoolExecutor(NCORES) as ex:
        list(
            ex.map(
                lambda c: _unshard_core(c, results[c]["outT"], outs), range(NCORES)
            )
        )
    h_last = np.ascontiguousarray(outs[:, -1, :])
    return outs, h_last


def _ensure_ntff_hook():
    """Install the axon NTFF profile hook if the image's antenv lacks it."""
    try:
        from antenv.axon_hooks import get_axon_ntff_profile_hook  # noqa: F401

        return True
    except ImportError:
        pass
    try:
        import types

        import antenv
        from trn_agent_boot.trn_boot import _ntff_profile_via_ctypes

        hook = _ntff_profile_via_ctypes("/opt/axon/libaxon_pjrt.so")
        mod = types.ModuleType("antenv.axon_hooks")
        mod.get_axon_ntff_profile_hook = lambda: hook
        mod.set_axon_ntff_profile_hook = lambda h: None
        sys.modules["antenv.axon_hooks"] = mod
        antenv.axon_hooks = mod
        return hook is not None
    except Exception as e:  # pragma: no cover
        print(f"NTFF hook install failed: {e}", file=sys.stderr)
        return False


def run(inputs, trace=False):
    """Run on hardware; returns ((outs, h_last), exec_time_ns_or_None)."""
    import concourse.bass_utils as bass_utils

    if trace:
        _ensure_ntff_hook()
        bass_utils.upload_artifacts = lambda tmpdir: f"local:{tmpdir}"

    nc = _get_built()
    in_maps = _prep_in_maps(inputs)
    res = bass_utils.run_bass_kernel_spmd(
        nc, in_maps, core_ids=list(range(NCORES)), trace=trace
    )
    return _assemble(res.results), res.exec_time_ns


def kernel(**inputs):
    (outs, h_last), _ = run(inputs, trace=False)
    return outs, h_last
